# revision 43
# baseline (speedup 1.0000x reference)
"""BiGCN (nn_BiGCN_52716428591487) Trainium2 kernel.

Math: the model's output is log_softmax(cat(l2_bu[root], l2_td[root]) @ W_lin + b).
Only the layer-2 GCN values AT THE ROOT NODES matter, and GCNConv is linear in
its input features, so the whole network collapses to:

  agg1_d[v]  = sum_{e -> v} coef_d(e) * x[nbr(e)]            (v in S; self-loops
               folded into the edge list with coef dinv_d[v]^2)
  l1_d[v]    = agg1_d[v] @ W_d1 + b_d1
  out2_l1[g] = sum_{s in S_g} Pr[s, g] * relu(l1_d[s])       (layer-2 agg)
  out2_R[g]  = c_g * relu(x[root_g])                          (root-feature block
               collapses: Pr is block-diagonal by graph, c_g = sum_s Pr[s, g])
  pb/pt[g]   = relu(W2_d^T [out2_R; out2_l1_d] + b_2)
  out[g]     = log_softmax([pb, pt][g] @ W_lin + b_lin)

where S = {sources of root-incident edges} + {roots} (~1.7k of 50k nodes) and
Pr is the (structure-only) layer-2 aggregation matrix.

Host does index-only preprocessing (degrees, edge selection, gather tables,
Pr); the device does every arithmetic op that touches x: the per-edge
coefficient scaling + aggregation (host-packed fp8 one-hot matrices,
DoubleRow fp8 matmuls on the PE), all four GCN feature transforms, biases,
relus, the linear head and log_softmax.

Sharding: graph-data parallel over 8 cores; the host concatenates the
per-core [G_cap, C] outputs.
"""

import numpy as np

P = 128
NCORES = 8
CH = 4  # xt pairs per DMA



def _roundup(a, m):
    return -(-int(a) // m) * m


# ----------------------------------------------------------------------------
# Host preprocessing: index-only work + gather tables
# ----------------------------------------------------------------------------

def _preprocess(x, edge_index, batch, num_graphs):
    import concourse.mybir as mybir
    import ml_dtypes

    f8 = mybir.dt.np(mybir.dt.float8e4)
    bf16 = ml_dtypes.bfloat16

    x = np.ascontiguousarray(np.asarray(x), dtype=np.float32)
    ei = np.asarray(edge_index)
    batch = np.asarray(batch).astype(np.int64)
    G = int(np.asarray(num_graphs))
    N, F = x.shape
    src = ei[0].astype(np.int64)
    dst = ei[1].astype(np.int64)

    assert np.all(np.diff(batch) >= 0), "batch must be sorted (contiguous graphs)"
    roots = np.searchsorted(batch, np.arange(G, dtype=np.int64))  # segment_min

    deg_td = 1.0 + np.bincount(dst, minlength=N).astype(np.float64)
    deg_bu = 1.0 + np.bincount(src, minlength=N).astype(np.float64)
    dinv_td = (1.0 / np.sqrt(deg_td)).astype(np.float32)
    dinv_bu = (1.0 / np.sqrt(deg_bu)).astype(np.float32)

    G_cap = max(-(-G // NCORES), 1)

    # S: sources of root-incident edges + roots
    is_root = np.zeros(N, bool)
    is_root[roots] = True
    rmask = is_root[dst]
    r_src, r_dst = src[rmask], dst[rmask]
    r_coef = dinv_td[r_src] * dinv_td[r_dst]

    s_nodes = np.unique(np.concatenate([r_src, roots]))  # sorted
    s_graph = batch[s_nodes]
    s_count_g = np.bincount(s_graph, minlength=G)
    assert s_count_g.max() <= P, "graph S-set exceeds one chunk"

    # entry weights per graph (edges at S nodes + self loops), per direction
    in_s = np.zeros(N, bool)
    in_s[s_nodes] = True
    gw_td = np.bincount(batch[dst[in_s[dst]]], minlength=G) + s_count_g
    gw_bu = np.bincount(batch[src[in_s[src]]], minlength=G) + s_count_g

    # graph -> core: greedy balance, cap G_cap
    core_of_graph = np.empty(G, np.int64)
    glocal = np.empty(G, np.int64)
    counts = np.zeros(NCORES, np.int64)
    ld_td = np.zeros(NCORES)
    ld_bu = np.zeros(NCORES)
    for g in np.argsort(-(gw_td + gw_bu), kind="stable"):
        c = min((cc for cc in range(NCORES) if counts[cc] < G_cap),
                key=lambda cc: max(ld_td[cc] + gw_td[g], ld_bu[cc] + gw_bu[g]))
        core_of_graph[g] = c
        glocal[g] = counts[c]
        counts[c] += 1
        ld_td[c] += gw_td[g]
        ld_bu[c] += gw_bu[g]

    s_core = core_of_graph[s_graph]
    S_counts = np.bincount(s_core, minlength=NCORES)
    S_cap = max(_roundup(S_counts.max(), P), P)
    nS = S_cap // P
    assert S_cap <= 512, f"S_cap={S_cap} > 512 unsupported"

    # within each core: whole graphs -> chunks (slot bins of P), balancing
    # entry load; then S nodes get consecutive slots inside their chunk
    s_lookup = np.full(N, -1, np.int64)
    chunk_of_graph = np.full(G, -1, np.int64)
    for c in range(NCORES):
        gs = np.flatnonzero(core_of_graph == c)
        fill = np.zeros(nS, np.int64)
        loads_td = np.zeros(nS)
        loads_bu = np.zeros(nS)
        for g in gs[np.argsort(-(gw_td[gs] + gw_bu[gs]), kind="stable")]:
            b = min((bb for bb in range(nS) if fill[bb] + s_count_g[g] <= P),
                    key=lambda bb: max(loads_td[bb] + gw_td[g],
                                       loads_bu[bb] + gw_bu[g]))
            chunk_of_graph[g] = b
            idx = s_nodes[s_graph == g]
            s_lookup[idx] = b * P + fill[b] + np.arange(len(idx))
            fill[b] += s_count_g[g]
            loads_td[b] += gw_td[g]
            loads_bu[b] += gw_bu[g]

    # entry lists per (core, dir): targets in S + self loops
    def _dir_entries(tgt_nodes, row_nodes, dinv):
        m = s_lookup[tgt_nodes] >= 0
        tgt = s_lookup[tgt_nodes[m]]
        rows = row_nodes[m]
        coef = dinv[row_nodes[m]] * dinv[tgt_nodes[m]]
        core = s_core[np.searchsorted(s_nodes, tgt_nodes[m])]
        # self loops
        tgt = np.concatenate([tgt, s_lookup[s_nodes]])
        rows = np.concatenate([rows, s_nodes])
        coef = np.concatenate([coef.astype(np.float32), dinv[s_nodes] ** 2])
        core = np.concatenate([core, s_core])
        return tgt, rows, coef, core

    dirs = {"td": _dir_entries(dst, src, dinv_td),
            "bu": _dir_entries(src, dst, dinv_bu)}

    # uniform k-tile count per chunk across cores/dirs, rounded to even
    Kc = 2
    for tgt, rows, coef, core in dirs.values():
        for c in range(NCORES):
            ch_arr = tgt[core == c] // P
            for s in range(nS):
                n = int(np.count_nonzero(ch_arr == s))
                Kc = max(Kc, _roundup(-(-n // P), 2))
    K = nS * Kc

    # layer-2 aggregation matrix Pr[core, s_local, glocal] and root tables
    r_graph = batch[r_dst]
    assert np.all(core_of_graph[batch[r_src]] == core_of_graph[r_graph]), \
        "cross-core root edge unsupported"
    Pr = np.zeros((NCORES, S_cap, G_cap), np.float32)
    np.add.at(Pr, (core_of_graph[r_graph], s_lookup[r_src], glocal[r_graph]), r_coef)
    np.add.at(Pr, (core_of_graph[np.arange(G)], s_lookup[roots], glocal),
              dinv_td[roots] ** 2)

    in_maps = []
    for c in range(NCORES):
        m = {"pr": Pr[c]}
        for name, (tgt, rows, coef, core) in dirs.items():
            sel = core == c
            tg_p = np.zeros(K * P, np.float32)
            cf_p = np.zeros(K * P, np.float32)
            rows_p = np.zeros(K * P, np.int64)
            tc_, rc_, cc_ = tgt[sel], rows[sel], coef[sel]
            ch_arr = tc_ // P
            for s in range(nS):
                ss = ch_arr == s
                n = int(np.count_nonzero(ss))
                # bu processes chunks in reverse so the two directions'
                # last-chunk tail chains stagger instead of bunching
                sk = (nS - 1 - s) if name == "bu" else s
                o = sk * Kc * P
                tg_p[o:o + n] = (tc_[ss] - s * P).astype(np.float32)
                cf_p[o:o + n] = cc_[ss]
                rows_p[o:o + n] = rc_[ss]
            # host-packed fp8 one-hot pair matrices [P, K*P]
            # (pair-major: col = pair*2P + h*P + tgt), one DMA per direction
            ohm = np.zeros((K, P, P), f8)
            tg_i = tg_p.reshape(K, P).astype(np.int64)
            ohm[np.repeat(np.arange(K), P), np.tile(np.arange(P), K),
                tg_i.reshape(-1)] = cf_p.reshape(-1).astype(f8)
            m[f"oh_{name}"] = np.ascontiguousarray(
                ohm.transpose(1, 0, 2).reshape(P, K * P))
            xg = x[rows_p].astype(f8)                    # [K*P, F]
            # pair-pack for DoubleRow: [K/2, 2, P, F] -> [K/2, P, 2F], then
            # group CH pairs per DMA block: [nDMA*P, CH*2F]
            PAIRS = K // 2
            nDMA = -(-PAIRS // CH)
            xp = np.zeros((nDMA * CH, P, 2 * F), f8)
            xp[:PAIRS] = xg.reshape(PAIRS, 2, P, F).transpose(0, 2, 1, 3) \
                           .reshape(PAIRS, P, 2 * F)
            xp = np.ascontiguousarray(
                xp.reshape(nDMA, CH, P, 2 * F).transpose(0, 2, 1, 3)
                  .reshape(nDMA * P, CH * 2 * F))
            m[f"xt_{name}"] = xp
        # root tables: xrootT chunks [P, G_cap] and c_g broadcast
        gs = np.flatnonzero(core_of_graph == c)
        xrootT = np.zeros((F, G_cap), np.float32)
        xrootT[:, glocal[gs]] = x[roots[gs]].T
        m["xrootT"] = xrootT
        m["croot"] = np.tile(Pr[c].sum(axis=0, dtype=np.float64)
                             .astype(np.float32), (P, 1))
        in_maps.append(m)

    meta = dict(F=F, S_cap=S_cap, K=K, Kc=Kc, G_cap=G_cap, counts=counts, G=G,
                core_of_graph=core_of_graph, glocal=glocal)
    return in_maps, meta


def _c16_layout(F, H, C, G_cap):
    """Column layout of the bf16 constant matrix [P, W16]."""
    off = 0
    L = {}

    def add(name, w):
        nonlocal off
        L[name] = (off, w)
        off += w

    for d in ("td", "bu"):
        for f in range(F // P):
            add(f"w1{d}{f}", H)
    for d in ("bu", "td"):
        for f in range((F + H) // P):
            add(f"w2{d}{f}", H)
    for f in range(2 * H // P):
        add(f"wl{f}", C)
    add("b1td", H)
    add("b1bu", H)
    add("bl", C)
    add("ones", P)
    for s in range(2):
        add(f"pr{s}", G_cap)
    return L, off


def _c32_layout(F, G_cap):
    off = 0
    L = {}

    def add(name, w):
        nonlocal off
        L[name] = (off, w)
        off += w

    for f in range(F // P):
        add(f"xrootT{f}", G_cap)
    add("croot", G_cap)
    add("b2bu", 1)
    add("b2td", 1)
    return L, off


def _pack_consts(in_maps, inputs, meta):
    """Fold per-core constants into cold16 [P,W16] bf16 / cold32 [P,W32] fp32."""
    import ml_dtypes

    bf16 = ml_dtypes.bfloat16
    H = int(np.asarray(inputs["W_td1"]).shape[1])
    C = int(np.asarray(inputs["W_lin"]).shape[1])
    F, K, G_cap, S_cap = meta["F"], meta["K"], meta["G_cap"], meta["S_cap"]
    nS = S_cap // P
    L16, W16 = _c16_layout(F, H, C, G_cap)
    L32, W32 = _c32_layout(F, G_cap)
    g = lambda k: np.asarray(inputs[k], dtype=np.float32)

    base16 = np.zeros((P, W16), bf16)

    def put(name, block):
        o, w = L16[name]
        base16[:, o:o + w][tuple(slice(s) for s in block.shape)] = \
            block.astype(bf16)

    for d, wn in (("td", "W_td1"), ("bu", "W_bu1")):
        for f in range(F // P):
            put(f"w1{d}{f}", g(wn)[f * P:(f + 1) * P, :])
    for d, wn in (("bu", "W_bu2"), ("td", "W_td2")):
        for f in range((F + H) // P):
            put(f"w2{d}{f}", g(wn)[f * P:(f + 1) * P, :])
    for f in range(2 * H // P):
        put(f"wl{f}", g("W_lin")[f * P:(f + 1) * P, :])
    put("b1td", g("b_td1").reshape(1, H))
    put("b1bu", g("b_bu1").reshape(1, H))
    put("bl", g("b_lin").reshape(1, C))
    put("ones", np.ones((1, P), np.float32))

    for m in in_maps:
        c16 = base16.copy()
        pr = m.pop("pr")
        for s in range(nS):
            o, w = L16[f"pr{s}"]
            c16[:, o:o + w] = pr[s * P:(s + 1) * P].astype(bf16)
        m["c16"] = np.ascontiguousarray(c16)

        c32 = np.zeros((P, W32), np.float32)
        xrootT = m.pop("xrootT")
        for f in range(F // P):
            o, w = L32[f"xrootT{f}"]
            c32[:, o:o + w] = xrootT[f * P:(f + 1) * P]
        o, w = L32["croot"]
        c32[:, o:o + w] = m.pop("croot")
        c32[:, L32["b2bu"][0]] = g("b_bu2")
        c32[:, L32["b2td"][0]] = g("b_td2")
        m["c32"] = np.ascontiguousarray(c32)

    meta["H"], meta["C"] = H, C
    meta["bz"] = all(not np.any(np.asarray(inputs[k]))
                     for k in ("b_td1", "b_bu1", "b_bu2", "b_td2", "b_lin"))
    return H


# ----------------------------------------------------------------------------
# Device program
# ----------------------------------------------------------------------------

def _build_program(F, H, C, S_cap, K, Kc, G_cap, bz=False, repeat=1):
    from contextlib import ExitStack

    import concourse.bacc as bacc
    from concourse.masks import make_identity
    import concourse.bass as bass  # noqa: F401
    import concourse.mybir as mybir
    import concourse.tile as tile

    dt = mybir.dt
    f32, bf, f8 = dt.float32, dt.bfloat16, dt.float8e4
    nF = F // P
    nS = S_cap // P
    nW2 = (F + H) // P
    PAIRS = K // 2
    PPC = Kc // 2                   # pairs per chunk
    nDMA = -(-PAIRS // CH)          # xt DMAs per direction
    assert K == nS * Kc and Kc % 2 == 0
    assert F % P == 0 and H == P and (F + H) % P == 0
    L16, W16 = _c16_layout(F, H, C, G_cap)
    L32, W32 = _c32_layout(F, G_cap)
    WH = P + 4 * K

    nc = bacc.Bacc("TRN2", target_bir_lowering=False, debug=False,
                   num_devices=NCORES)

    xt_d = {d: nc.dram_tensor(f"xt_{d}", [nDMA * P, CH * 2 * F], f8,
                              kind="ExternalInput").ap() for d in ("td", "bu")}
    oh_d = {d: nc.dram_tensor(f"oh_{d}", [P, K * P], f8,
                              kind="ExternalInput").ap() for d in ("td", "bu")}
    c16_d = nc.dram_tensor("c16", [P, W16], bf, kind="ExternalInput").ap()
    c32_d = nc.dram_tensor("c32", [P, W32], f32, kind="ExternalInput").ap()
    out_d = nc.dram_tensor("out", [G_cap, C], f32, kind="ExternalOutput").ap()

    eq, mul, sub, addop, maxop = (
        mybir.AluOpType.is_equal, mybir.AluOpType.mult,
        mybir.AluOpType.subtract, mybir.AluOpType.add, mybir.AluOpType.max)
    Relu, Exp, Ln, Copy = (mybir.ActivationFunctionType.Relu,
                           mybir.ActivationFunctionType.Exp,
                           mybir.ActivationFunctionType.Ln,
                           mybir.ActivationFunctionType.Copy)
    DR = mybir.MatmulPerfMode.DoubleRow

    with ExitStack() as ctx:
        tc = ctx.enter_context(tile.TileContext(nc))
        fixed = ctx.enter_context(tc.tile_pool(name="fx", bufs=1))
        const = ctx.enter_context(tc.tile_pool(
            name="cst", bufs=(1 if repeat == 1 else 2)))
        xpool = ctx.enter_context(tc.tile_pool(name="xp", bufs=8))
        ohpool = ctx.enter_context(tc.tile_pool(name="oh", bufs=2))
        apool = ctx.enter_context(tc.tile_pool(name="ap", bufs=4))
        tpool = ctx.enter_context(tc.tile_pool(name="tp", bufs=2))
        spool = ctx.enter_context(tc.tile_pool(name="sp", bufs=2))
        psA = ctx.enter_context(tc.tile_pool(name="psA", bufs=1, space="PSUM"))
        psB = ctx.enter_context(tc.tile_pool(name="psB", bufs=4, space="PSUM"))

        ident = fixed.tile([P, P], bf, name="ident", tag="ident")
        make_identity(nc, ident[:])
        # load the one act table containing Exp/Ln/Relu/Copy up-front
        # (overlaps the initial DMA waits); the insert_act_table_loads pass
        # then has nothing to add mid-program
        from concourse.hw_specs import get_activation_tables
        need = {Exp, Ln, Relu, Copy}
        for set_id, funcs in enumerate(get_activation_tables(nc.m.arch).values()):
            if need <= funcs:
                nc.scalar.add_instruction(mybir.InstLoadActFuncSet(
                    name=nc.get_next_instruction_name(),
                    act_func_set_id=set_id, ins=[], outs=[]))
                break

        for _rep in range(repeat):
            c16 = const.tile([P, W16], bf, name="c16", tag="c16")
            c32 = const.tile([P, W32], f32, name="c32", tag="c32")

            def C16(name, rows=None):
                o, w = L16[name]
                return c16[:, o:o + w] if rows is None else c16[rows, o:o + w]

            def C32(name, rows=None):
                o, w = L32[name]
                return c32[:, o:o + w] if rows is None else c32[rows, o:o + w]

            # one-hot matrices (one DMA per direction) + xt stream
            ohm = {}
            xtiles = {"td": [], "bu": []}
            for d in ("td", "bu"):
                ohm[d] = ohpool.tile([P, K * P], f8, name=f"ohm{d}",
                                     tag=f"ohm{d}")
                nc.sync.dma_start(ohm[d][:], oh_d[d][:, :])
            for ci in range(nDMA):
                if ci == (nDMA + 1) // 2:
                    nc.sync.dma_start(c16[:], c16_d[:, :])
                    nc.sync.dma_start(c32[:], c32_d[:, :])
                for d in ("td", "bu"):
                    npair = min(CH, PAIRS - ci * CH)
                    t = xpool.tile([P, CH * 2 * F], f8, name="xt", tag="xt")
                    nc.sync.dma_start(
                        t[:, :npair * 2 * F],
                        xt_d[d][ci * P:(ci + 1) * P, :npair * 2 * F])
                    xtiles[d].append(t)

            # stage 1: DoubleRow fp8 matmuls with host-packed one-hot pairs
            agg_ps = {d: [psA.tile([P, F], f32, name=f"agg{d}{s}",
                                   tag=f"agg{d}{s}") for s in range(nS)]
                      for d in ("td", "bu")}
            done = []   # (d, s) chunks completed, in order
            for ci in range(nDMA):
                for d in ("td", "bu"):
                    npair = min(CH, PAIRS - ci * CH)
                    for j in range(npair):
                        pr_i = ci * CH + j
                        s = pr_i // PPC
                        if d == "bu":
                            s = nS - 1 - s
                        nc.tensor.matmul(
                            out=agg_ps[d][s][:],
                            lhsT=ohm[d][:, pr_i * 2 * P:(pr_i + 1) * 2 * P]
                                .rearrange("p (a b) -> p a b", a=2),
                            rhs=xtiles[d][ci][:, j * 2 * F:(j + 1) * 2 * F]
                                .rearrange("p (a b) -> p a b", a=2),
                            start=(pr_i % PPC == 0),
                            stop=(pr_i % PPC == PPC - 1),
                            perf_mode=DR)
                        if pr_i % PPC == PPC - 1:
                            done.append((d, s))

            # stage 1.5 + 2, per completed (d, s) chunk:
            # psum -> sbuf (Act), transpose (PE), copy back (Act),
            # l1 = aggT^T @ W1 + b1 (PE), relu -> cbt (DVE)
            aggT = {d: [tpool.tile([P, S_cap], bf, name=f"aT{d}{f}",
                                   tag=f"aT{d}{f}") for f in range(nF)]
                    for d in ("td", "bu")}
            cbt = [spool.tile([P, 2 * H], bf, name=f"cbt{s}", tag=f"cbt{s}")
                   for s in range(nS)]
            DI = {"bu": 0, "td": 1}
            for d, s in done:
                # the last chunks' chains are tail-critical: run one on DVE
                # (idle after the one-hot builds) so they parallelize with
                # the Activation-engine chain of the other
                on_dve = s == nS - 1 and d == "td"

                def _copy(dst, src_ap):
                    if on_dve:
                        nc.vector.tensor_scalar(out=dst, in0=src_ap,
                                                scalar1=0.0, scalar2=None,
                                                op0=addop)
                    else:
                        nc.scalar.activation(dst, src_ap, Copy)

                aggS = apool.tile([P, F], bf, name="aggS", tag="aggS")
                _copy(aggS[:], agg_ps[d][s][:])
                for f in range(nF):
                    tps = psB.tile([P, P], bf, name="tps", tag="psb")
                    nc.tensor.transpose(out=tps[:],
                                        in_=aggS[:, f * P:(f + 1) * P],
                                        identity=ident[:])
                    _copy(aggT[d][f][:, s * P:(s + 1) * P], tps[:])
                h = psB.tile([P, H], f32, name="hps", tag="psb")
                for f in range(nF):
                    nc.tensor.matmul(out=h[:],
                                     lhsT=aggT[d][f][:, s * P:(s + 1) * P],
                                     rhs=C16(f"w1{d}{f}"),
                                     start=(f == 0),
                                     stop=(bz and f == nF - 1))
                if not bz:
                    nc.tensor.matmul(out=h[:],
                                     lhsT=C16("ones", rows=slice(0, 1)),
                                     rhs=C16(f"b1{d}", rows=slice(0, 1)),
                                     start=False, stop=True)
                di = DI[d]
                nc.vector.tensor_scalar(out=cbt[s][:, di * H:(di + 1) * H],
                                        in0=h[:], scalar1=0.0, scalar2=None,
                                        op0=maxop)

            # root block: out2_R = relu(xrootT) * c_g
            rT = []
            for f in range(nF):
                t = spool.tile([P, G_cap], bf, name=f"rT{f}", tag=f"rT{f}")
                tmp = spool.tile([P, G_cap], f32, name="rtmp", tag="rtmp")
                nc.scalar.activation(tmp[:], C32(f"xrootT{f}"), Relu)
                nc.vector.tensor_tensor(out=t[:], in0=tmp[:], in1=C32("croot"),
                                        op=mul)
                rT.append(t)

            # stage 4: o2[m] [P, G_cap] = cbt[:, m-chunk]^T @ Pr
            o2_sb = []
            for m_ in range(2 * H // P):
                o2 = psB.tile([P, G_cap], f32, name="o2ps", tag="psb")
                for s in range(nS):
                    nc.tensor.matmul(out=o2[:],
                                     lhsT=cbt[s][:, m_ * P:(m_ + 1) * P],
                                     rhs=C16(f"pr{s}"), start=(s == 0),
                                     stop=(s == nS - 1))
                t = spool.tile([P, G_cap], bf, name=f"o2{m_}", tag=f"o2{m_}")
                nc.vector.tensor_scalar(out=t[:], in0=o2[:], scalar1=0.0,
                                        scalar2=None, op0=addop)
                o2_sb.append(t)

            # stage 5: totT[d] [H, G_cap] = relu(W2_d^T [rT; o2_d] + b2_d)
            tot = []
            for di, d in enumerate(("bu", "td")):
                tp = psB.tile([P, G_cap], f32, name="totps", tag="psb")
                for f in range(nW2):
                    rhs_t = rT[f] if f < nF else o2_sb[di]
                    nc.tensor.matmul(out=tp[:], lhsT=C16(f"w2{d}{f}"),
                                     rhs=rhs_t[:], start=(f == 0),
                                     stop=(f == nW2 - 1))
                t = spool.tile([P, G_cap], bf, name=f"tot{di}", tag=f"tot{di}")
                nc.vector.tensor_scalar(out=t[:], in0=tp[:],
                                        scalar1=C32(f"b2{d}"), scalar2=0.0,
                                        op0=addop, op1=maxop)
                tot.append(t)

            # stage 6: logits + log_softmax
            lg = psB.tile([G_cap, C], f32, name="lgps", tag="psb")
            nwl = 2 * H // P
            for f in range(nwl):
                nc.tensor.matmul(out=lg[:], lhsT=tot[f][:, :G_cap],
                                 rhs=C16(f"wl{f}"), start=(f == 0),
                                 stop=(bz and f == nwl - 1))
            if not bz:
                nc.tensor.matmul(out=lg[:],
                                 lhsT=C16("ones", rows=slice(0, 1))[:, :G_cap],
                                 rhs=C16("bl", rows=slice(0, 1)),
                                 start=False, stop=True)
            ez = spool.tile([G_cap, C], f32, name="ez", tag="ez")
            se = spool.tile([G_cap, 1], f32, name="se", tag="se")
            nc.scalar.activation(ez[:], lg[:], Exp, accum_out=se[:])
            lse = spool.tile([G_cap, 1], f32, name="lse", tag="lse")
            nc.scalar.activation(lse[:], se[:], Ln)
            res = spool.tile([G_cap, C], f32, name="res", tag="res")
            nc.vector.tensor_scalar(out=res[:], in0=lg[:], scalar1=lse[:],
                                    scalar2=None, op0=sub)
            nc.sync.dma_start(out_d[:], res[:])

    nc.compile()
    return nc


_PROG_CACHE = {}


def _prepare_maps(inputs):
    in_maps, meta = _preprocess(inputs["x"], inputs["edge_index"],
                                inputs["batch"], inputs["num_graphs"])
    _pack_consts(in_maps, inputs, meta)
    return in_maps, meta


def _prepare(inputs):
    in_maps, meta = _prepare_maps(inputs)
    key = (meta["F"], meta["H"], meta["C"], meta["S_cap"], meta["K"],
           meta["Kc"], meta["G_cap"], meta["bz"])
    if key not in _PROG_CACHE:
        _PROG_CACHE[key] = _build_program(*key)
    return _PROG_CACHE[key], in_maps, meta


def kernel(**inputs):
    from concourse.bass_utils import run_bass_kernel_spmd

    nc, in_maps, meta = _prepare(inputs)
    res = run_bass_kernel_spmd(nc, in_maps, list(range(NCORES)))
    G = meta["G"]
    cog, gl = meta["core_of_graph"], meta["glocal"]
    out = np.empty((G, meta["C"]), np.float32)
    for g in range(G):
        out[g] = res.results[cog[g]]["out"][gl[g]]
    return out


# revision 45
# speedup vs baseline: 1.0865x; 1.0865x over previous
"""BiGCN (nn_BiGCN_52716428591487) Trainium2 kernel.

Math: the model's output is log_softmax(cat(l2_bu[root], l2_td[root]) @ W_lin + b).
Only the layer-2 GCN values AT THE ROOT NODES matter, and GCNConv is linear in
its input features, so the whole network collapses to:

  agg1_d[v]  = sum_{e -> v} coef_d(e) * x[nbr(e)]            (v in S; self-loops
               folded into the edge list with coef dinv_d[v]^2)
  l1_d[v]    = agg1_d[v] @ W_d1 + b_d1
  out2_l1[g] = sum_{s in S_g} Pr[s, g] * relu(l1_d[s])       (layer-2 agg)
  out2_R[g]  = c_g * relu(x[root_g])                          (root-feature block
               collapses: Pr is block-diagonal by graph, c_g = sum_s Pr[s, g])
  pb/pt[g]   = relu(W2_d^T [out2_R; out2_l1_d] + b_2)
  out[g]     = log_softmax([pb, pt][g] @ W_lin + b_lin)

where S = {sources of root-incident edges} + {roots} (~1.7k of 50k nodes) and
Pr is the (structure-only) layer-2 aggregation matrix.

Host does index-only preprocessing (degrees, edge selection, gather tables,
Pr); the device does every arithmetic op that touches x: the per-edge
coefficient scaling + aggregation (host-packed fp8 one-hot matrices,
DoubleRow fp8 matmuls on the PE), all four GCN feature transforms, biases,
relus, the linear head and log_softmax.

Sharding: graph-data parallel over 8 cores; the host concatenates the
per-core [G_cap, C] outputs.
"""

import numpy as np

P = 128
NCORES = 8
CH = 4  # xt pairs per DMA



def _roundup(a, m):
    return -(-int(a) // m) * m


# ----------------------------------------------------------------------------
# Host preprocessing: index-only work + gather tables
# ----------------------------------------------------------------------------

def _preprocess(x, edge_index, batch, num_graphs):
    import concourse.mybir as mybir
    import ml_dtypes

    f8 = mybir.dt.np(mybir.dt.float8e4)
    bf16 = ml_dtypes.bfloat16

    x = np.ascontiguousarray(np.asarray(x), dtype=np.float32)
    ei = np.asarray(edge_index)
    batch = np.asarray(batch).astype(np.int64)
    G = int(np.asarray(num_graphs))
    N, F = x.shape
    src = ei[0].astype(np.int64)
    dst = ei[1].astype(np.int64)

    assert np.all(np.diff(batch) >= 0), "batch must be sorted (contiguous graphs)"
    roots = np.searchsorted(batch, np.arange(G, dtype=np.int64))  # segment_min

    deg_td = 1.0 + np.bincount(dst, minlength=N).astype(np.float64)
    deg_bu = 1.0 + np.bincount(src, minlength=N).astype(np.float64)
    dinv_td = (1.0 / np.sqrt(deg_td)).astype(np.float32)
    dinv_bu = (1.0 / np.sqrt(deg_bu)).astype(np.float32)

    G_cap = max(-(-G // NCORES), 1)

    # S: sources of root-incident edges + roots
    is_root = np.zeros(N, bool)
    is_root[roots] = True
    rmask = is_root[dst]
    r_src, r_dst = src[rmask], dst[rmask]
    r_coef = dinv_td[r_src] * dinv_td[r_dst]

    s_nodes = np.unique(np.concatenate([r_src, roots]))  # sorted
    s_graph = batch[s_nodes]
    s_count_g = np.bincount(s_graph, minlength=G)
    assert s_count_g.max() <= P, "graph S-set exceeds one chunk"

    # entry weights per graph (edges at S nodes + self loops), per direction
    in_s = np.zeros(N, bool)
    in_s[s_nodes] = True
    gw_td = np.bincount(batch[dst[in_s[dst]]], minlength=G) + s_count_g
    gw_bu = np.bincount(batch[src[in_s[src]]], minlength=G) + s_count_g

    # graph -> core: greedy balance, cap G_cap
    core_of_graph = np.empty(G, np.int64)
    glocal = np.empty(G, np.int64)
    counts = np.zeros(NCORES, np.int64)
    ld_td = np.zeros(NCORES)
    ld_bu = np.zeros(NCORES)
    for g in np.argsort(-(gw_td + gw_bu), kind="stable"):
        c = min((cc for cc in range(NCORES) if counts[cc] < G_cap),
                key=lambda cc: max(ld_td[cc] + gw_td[g], ld_bu[cc] + gw_bu[g]))
        core_of_graph[g] = c
        glocal[g] = counts[c]
        counts[c] += 1
        ld_td[c] += gw_td[g]
        ld_bu[c] += gw_bu[g]

    s_core = core_of_graph[s_graph]
    S_counts = np.bincount(s_core, minlength=NCORES)
    S_cap = max(_roundup(S_counts.max(), P), P)
    nS = S_cap // P
    assert S_cap <= 512, f"S_cap={S_cap} > 512 unsupported"

    # within each core: whole graphs -> chunks (slot bins of P), balancing
    # entry load; then S nodes get consecutive slots inside their chunk
    s_lookup = np.full(N, -1, np.int64)
    chunk_of_graph = np.full(G, -1, np.int64)
    for c in range(NCORES):
        gs = np.flatnonzero(core_of_graph == c)
        fill = np.zeros(nS, np.int64)
        loads_td = np.zeros(nS)
        loads_bu = np.zeros(nS)
        for g in gs[np.argsort(-(gw_td[gs] + gw_bu[gs]), kind="stable")]:
            b = min((bb for bb in range(nS) if fill[bb] + s_count_g[g] <= P),
                    key=lambda bb: max(loads_td[bb] + gw_td[g],
                                       loads_bu[bb] + gw_bu[g]))
            chunk_of_graph[g] = b
            idx = s_nodes[s_graph == g]
            s_lookup[idx] = b * P + fill[b] + np.arange(len(idx))
            fill[b] += s_count_g[g]
            loads_td[b] += gw_td[g]
            loads_bu[b] += gw_bu[g]

    # entry lists per (core, dir): targets in S + self loops
    def _dir_entries(tgt_nodes, row_nodes, dinv):
        m = s_lookup[tgt_nodes] >= 0
        tgt = s_lookup[tgt_nodes[m]]
        rows = row_nodes[m]
        coef = dinv[row_nodes[m]] * dinv[tgt_nodes[m]]
        core = s_core[np.searchsorted(s_nodes, tgt_nodes[m])]
        # self loops
        tgt = np.concatenate([tgt, s_lookup[s_nodes]])
        rows = np.concatenate([rows, s_nodes])
        coef = np.concatenate([coef.astype(np.float32), dinv[s_nodes] ** 2])
        core = np.concatenate([core, s_core])
        return tgt, rows, coef, core

    dirs = {"td": _dir_entries(dst, src, dinv_td),
            "bu": _dir_entries(src, dst, dinv_bu)}

    # uniform k-tile count per chunk across cores/dirs, rounded to even
    Kc = 2
    for tgt, rows, coef, core in dirs.values():
        for c in range(NCORES):
            ch_arr = tgt[core == c] // P
            for s in range(nS):
                n = int(np.count_nonzero(ch_arr == s))
                Kc = max(Kc, _roundup(-(-n // P), 2))
    K = nS * Kc

    # layer-2 aggregation matrix Pr[core, s_local, glocal] and root tables
    r_graph = batch[r_dst]
    assert np.all(core_of_graph[batch[r_src]] == core_of_graph[r_graph]), \
        "cross-core root edge unsupported"
    Pr = np.zeros((NCORES, S_cap, G_cap), np.float32)
    np.add.at(Pr, (core_of_graph[r_graph], s_lookup[r_src], glocal[r_graph]), r_coef)
    np.add.at(Pr, (core_of_graph[np.arange(G)], s_lookup[roots], glocal),
              dinv_td[roots] ** 2)

    in_maps = []
    for c in range(NCORES):
        m = {"pr": Pr[c]}
        for name, (tgt, rows, coef, core) in dirs.items():
            sel = core == c
            tg_p = np.zeros(K * P, np.float32)
            cf_p = np.zeros(K * P, np.float32)
            rows_p = np.zeros(K * P, np.int64)
            tc_, rc_, cc_ = tgt[sel], rows[sel], coef[sel]
            ch_arr = tc_ // P
            for s in range(nS):
                ss = ch_arr == s
                n = int(np.count_nonzero(ss))
                # bu processes chunks in reverse so the two directions'
                # last-chunk tail chains stagger instead of bunching
                sk = (nS - 1 - s) if name == "bu" else s
                o = sk * Kc * P
                tg_p[o:o + n] = (tc_[ss] - s * P).astype(np.float32)
                cf_p[o:o + n] = cc_[ss]
                rows_p[o:o + n] = rc_[ss]
            # host-packed fp8 one-hot pair matrices [P, K*P]
            # (pair-major: col = pair*2P + h*P + tgt), one DMA per direction
            ohm = np.zeros((K, P, P), f8)
            tg_i = tg_p.reshape(K, P).astype(np.int64)
            ohm[np.repeat(np.arange(K), P), np.tile(np.arange(P), K),
                tg_i.reshape(-1)] = cf_p.reshape(-1).astype(f8)
            m[f"oh_{name}"] = np.ascontiguousarray(
                ohm.transpose(1, 0, 2).reshape(P, K * P))
            xg = x[rows_p].astype(f8)                    # [K*P, F]
            # pair-pack for DoubleRow: [K/2, 2, P, F] -> [K/2, P, 2F], then
            # group CH pairs per DMA block: [nDMA*P, CH*2F]
            PAIRS = K // 2
            nDMA = -(-PAIRS // CH)
            xp = np.zeros((nDMA * CH, P, 2 * F), f8)
            xp[:PAIRS] = xg.reshape(PAIRS, 2, P, F).transpose(0, 2, 1, 3) \
                           .reshape(PAIRS, P, 2 * F)
            xp = np.ascontiguousarray(
                xp.reshape(nDMA, CH, P, 2 * F).transpose(0, 2, 1, 3)
                  .reshape(nDMA * P, CH * 2 * F))
            m[f"xt_{name}"] = xp
        # root tables: xrootT chunks [P, G_cap] and c_g broadcast
        gs = np.flatnonzero(core_of_graph == c)
        xrootT = np.zeros((F, G_cap), np.float32)
        xrootT[:, glocal[gs]] = x[roots[gs]].T
        m["xrootT"] = xrootT
        m["croot"] = np.tile(Pr[c].sum(axis=0, dtype=np.float64)
                             .astype(np.float32), (P, 1))
        in_maps.append(m)

    meta = dict(F=F, S_cap=S_cap, K=K, Kc=Kc, G_cap=G_cap, counts=counts, G=G,
                core_of_graph=core_of_graph, glocal=glocal)
    return in_maps, meta


def _c16_layout(F, H, C, G_cap):
    """Column layout of the bf16 constant matrix [P, W16]."""
    off = 0
    L = {}

    def add(name, w):
        nonlocal off
        L[name] = (off, w)
        off += w

    for d in ("td", "bu"):
        for f in range(F // P):
            add(f"w1{d}{f}", H)
    for d in ("bu", "td"):
        for f in range((F + H) // P):
            add(f"w2{d}{f}", H)
    for f in range(2 * H // P):
        add(f"wl{f}", C)
    add("b1td", H)
    add("b1bu", H)
    add("bl", C)
    add("ones", P)
    for s in range(2):
        add(f"pr{s}", G_cap)
    return L, off


def _c32_layout(F, G_cap):
    off = 0
    L = {}

    def add(name, w):
        nonlocal off
        L[name] = (off, w)
        off += w

    for f in range(F // P):
        add(f"xrootT{f}", G_cap)
    add("croot", G_cap)
    add("b2bu", 1)
    add("b2td", 1)
    return L, off


def _pack_consts(in_maps, inputs, meta):
    """Fold per-core constants into cold16 [P,W16] bf16 / cold32 [P,W32] fp32."""
    import ml_dtypes

    bf16 = ml_dtypes.bfloat16
    H = int(np.asarray(inputs["W_td1"]).shape[1])
    C = int(np.asarray(inputs["W_lin"]).shape[1])
    F, K, G_cap, S_cap = meta["F"], meta["K"], meta["G_cap"], meta["S_cap"]
    nS = S_cap // P
    L16, W16 = _c16_layout(F, H, C, G_cap)
    L32, W32 = _c32_layout(F, G_cap)
    g = lambda k: np.asarray(inputs[k], dtype=np.float32)

    base16 = np.zeros((P, W16), bf16)

    def put(name, block):
        o, w = L16[name]
        base16[:, o:o + w][tuple(slice(s) for s in block.shape)] = \
            block.astype(bf16)

    for d, wn in (("td", "W_td1"), ("bu", "W_bu1")):
        for f in range(F // P):
            put(f"w1{d}{f}", g(wn)[f * P:(f + 1) * P, :])
    for d, wn in (("bu", "W_bu2"), ("td", "W_td2")):
        for f in range((F + H) // P):
            put(f"w2{d}{f}", g(wn)[f * P:(f + 1) * P, :])
    for f in range(2 * H // P):
        put(f"wl{f}", g("W_lin")[f * P:(f + 1) * P, :])
    put("b1td", g("b_td1").reshape(1, H))
    put("b1bu", g("b_bu1").reshape(1, H))
    put("bl", g("b_lin").reshape(1, C))
    put("ones", np.ones((1, P), np.float32))

    for m in in_maps:
        c16 = base16.copy()
        pr = m.pop("pr")
        for s in range(nS):
            o, w = L16[f"pr{s}"]
            c16[:, o:o + w] = pr[s * P:(s + 1) * P].astype(bf16)
        m["c16"] = np.ascontiguousarray(c16)

        c32 = np.zeros((P, W32), np.float32)
        xrootT = m.pop("xrootT")
        for f in range(F // P):
            o, w = L32[f"xrootT{f}"]
            c32[:, o:o + w] = xrootT[f * P:(f + 1) * P]
        o, w = L32["croot"]
        c32[:, o:o + w] = m.pop("croot")
        c32[:, L32["b2bu"][0]] = g("b_bu2")
        c32[:, L32["b2td"][0]] = g("b_td2")
        m["c32"] = np.ascontiguousarray(c32)

    meta["H"], meta["C"] = H, C
    meta["bz"] = all(not np.any(np.asarray(inputs[k]))
                     for k in ("b_td1", "b_bu1", "b_bu2", "b_td2", "b_lin"))
    return H


# ----------------------------------------------------------------------------
# Device program
# ----------------------------------------------------------------------------

def _build_program(F, H, C, S_cap, K, Kc, G_cap, bz=False, repeat=1):
    from contextlib import ExitStack

    import concourse.bacc as bacc
    from concourse.masks import make_identity
    import concourse.bass as bass  # noqa: F401
    import concourse.mybir as mybir
    import concourse.tile as tile

    dt = mybir.dt
    f32, bf, f8 = dt.float32, dt.bfloat16, dt.float8e4
    nF = F // P
    nS = S_cap // P
    nW2 = (F + H) // P
    PAIRS = K // 2
    PPC = Kc // 2                   # pairs per chunk
    nDMA = -(-PAIRS // CH)          # xt DMAs per direction
    assert K == nS * Kc and Kc % 2 == 0
    assert F % P == 0 and H == P and (F + H) % P == 0
    L16, W16 = _c16_layout(F, H, C, G_cap)
    L32, W32 = _c32_layout(F, G_cap)
    WH = P + 4 * K

    nc = bacc.Bacc("TRN2", target_bir_lowering=False, debug=False,
                   num_devices=NCORES)

    xt_d = {d: nc.dram_tensor(f"xt_{d}", [nDMA * P, CH * 2 * F], f8,
                              kind="ExternalInput").ap() for d in ("td", "bu")}
    oh_d = {d: nc.dram_tensor(f"oh_{d}", [P, K * P], f8,
                              kind="ExternalInput").ap() for d in ("td", "bu")}
    c16_d = nc.dram_tensor("c16", [P, W16], bf, kind="ExternalInput").ap()
    c32_d = nc.dram_tensor("c32", [P, W32], f32, kind="ExternalInput").ap()
    out_d = nc.dram_tensor("out", [G_cap, C], f32, kind="ExternalOutput").ap()

    eq, mul, sub, addop, maxop = (
        mybir.AluOpType.is_equal, mybir.AluOpType.mult,
        mybir.AluOpType.subtract, mybir.AluOpType.add, mybir.AluOpType.max)
    Relu, Exp, Ln, Copy = (mybir.ActivationFunctionType.Relu,
                           mybir.ActivationFunctionType.Exp,
                           mybir.ActivationFunctionType.Ln,
                           mybir.ActivationFunctionType.Copy)
    DR = mybir.MatmulPerfMode.DoubleRow

    with ExitStack() as ctx:
        tc = ctx.enter_context(tile.TileContext(nc))
        fixed = ctx.enter_context(tc.tile_pool(name="fx", bufs=1))
        const = ctx.enter_context(tc.tile_pool(
            name="cst", bufs=(1 if repeat == 1 else 2)))
        xpool = ctx.enter_context(tc.tile_pool(name="xp", bufs=8))
        ohpool = ctx.enter_context(tc.tile_pool(name="oh", bufs=2))
        apool = ctx.enter_context(tc.tile_pool(name="ap", bufs=4))
        tpool = ctx.enter_context(tc.tile_pool(name="tp", bufs=2))
        spool = ctx.enter_context(tc.tile_pool(name="sp", bufs=2))
        psA = ctx.enter_context(tc.tile_pool(name="psA", bufs=1, space="PSUM"))
        psB = ctx.enter_context(tc.tile_pool(name="psB", bufs=4, space="PSUM"))

        ident = fixed.tile([P, P], bf, name="ident", tag="ident")
        make_identity(nc, ident[:])
        # load the one act table containing Exp/Ln/Relu/Copy up-front
        # (overlaps the initial DMA waits); the insert_act_table_loads pass
        # then has nothing to add mid-program
        from concourse.hw_specs import get_activation_tables
        need = {Exp, Ln, Relu, Copy}
        for set_id, funcs in enumerate(get_activation_tables(nc.m.arch).values()):
            if need <= funcs:
                nc.scalar.add_instruction(mybir.InstLoadActFuncSet(
                    name=nc.get_next_instruction_name(),
                    act_func_set_id=set_id, ins=[], outs=[]))
                break

        for _rep in range(repeat):
            c16 = const.tile([P, W16], bf, name="c16", tag="c16")
            c32 = const.tile([P, W32], f32, name="c32", tag="c32")

            def C16(name, rows=None):
                o, w = L16[name]
                return c16[:, o:o + w] if rows is None else c16[rows, o:o + w]

            def C32(name, rows=None):
                o, w = L32[name]
                return c32[:, o:o + w] if rows is None else c32[rows, o:o + w]

            # one-hot matrices (one DMA per direction) + xt stream
            ohm = {}
            xtiles = {"td": [], "bu": []}
            for d in ("td", "bu"):
                ohm[d] = ohpool.tile([P, K * P], f8, name=f"ohm{d}",
                                     tag=f"ohm{d}")
                nc.sync.dma_start(ohm[d][:], oh_d[d][:, :])
            for ci in range(nDMA):
                if ci == (nDMA + 1) // 2:
                    nc.sync.dma_start(c16[:], c16_d[:, :])
                    nc.sync.dma_start(c32[:], c32_d[:, :])
                for d in ("td", "bu"):
                    npair = min(CH, PAIRS - ci * CH)
                    t = xpool.tile([P, CH * 2 * F], f8, name="xt", tag="xt")
                    nc.sync.dma_start(
                        t[:, :npair * 2 * F],
                        xt_d[d][ci * P:(ci + 1) * P, :npair * 2 * F])
                    xtiles[d].append(t)

            # stage 1: DoubleRow fp8 matmuls with host-packed one-hot pairs
            agg_ps = {d: [psA.tile([P, F], f32, name=f"agg{d}{s}",
                                   tag=f"agg{d}{s}") for s in range(nS)]
                      for d in ("td", "bu")}
            done = []   # (d, s) chunks completed, in order
            for ci in range(nDMA):
                for d in ("td", "bu"):
                    npair = min(CH, PAIRS - ci * CH)
                    for j in range(npair):
                        pr_i = ci * CH + j
                        s = pr_i // PPC
                        if d == "bu":
                            s = nS - 1 - s
                        nc.tensor.matmul(
                            out=agg_ps[d][s][:],
                            lhsT=ohm[d][:, pr_i * 2 * P:(pr_i + 1) * 2 * P]
                                .rearrange("p (a b) -> p a b", a=2),
                            rhs=xtiles[d][ci][:, j * 2 * F:(j + 1) * 2 * F]
                                .rearrange("p (a b) -> p a b", a=2),
                            start=(pr_i % PPC == 0),
                            stop=(pr_i % PPC == PPC - 1),
                            perf_mode=DR)
                        if pr_i % PPC == PPC - 1:
                            done.append((d, s))

            # stage 1.5 + 2, per completed (d, s) chunk:
            # psum -> sbuf (Act), transpose (PE), copy back (Act),
            # l1 = aggT^T @ W1 + b1 (PE), relu -> cbt (DVE)
            aggT = {d: [tpool.tile([P, S_cap], bf, name=f"aT{d}{f}",
                                   tag=f"aT{d}{f}") for f in range(nF)]
                    for d in ("td", "bu")}
            cbt = [spool.tile([P, 2 * H], bf, name=f"cbt{s}", tag=f"cbt{s}")
                   for s in range(nS)]
            DI = {"bu": 0, "td": 1}
            for d, s in done:
                # the last chunks' chains are tail-critical: run one on DVE
                # (idle after the one-hot builds) so they parallelize with
                # the Activation-engine chain of the other
                on_dve = s == nS - 1 and d == "td"

                def _copy(dst, src_ap):
                    if on_dve:
                        nc.vector.tensor_scalar(out=dst, in0=src_ap,
                                                scalar1=0.0, scalar2=None,
                                                op0=addop)
                    else:
                        nc.scalar.activation(dst, src_ap, Copy)

                aggS = apool.tile([P, F], bf, name="aggS", tag="aggS")
                _copy(aggS[:], agg_ps[d][s][:])
                for f in range(nF):
                    tps = psB.tile([P, P], bf, name="tps", tag="psb")
                    nc.tensor.transpose(out=tps[:],
                                        in_=aggS[:, f * P:(f + 1) * P],
                                        identity=ident[:])
                    _copy(aggT[d][f][:, s * P:(s + 1) * P], tps[:])
                h = psB.tile([P, H], f32, name="hps", tag="psb")
                for f in range(nF):
                    nc.tensor.matmul(out=h[:],
                                     lhsT=aggT[d][f][:, s * P:(s + 1) * P],
                                     rhs=C16(f"w1{d}{f}"),
                                     start=(f == 0),
                                     stop=(bz and f == nF - 1))
                if not bz:
                    nc.tensor.matmul(out=h[:],
                                     lhsT=C16("ones", rows=slice(0, 1)),
                                     rhs=C16(f"b1{d}", rows=slice(0, 1)),
                                     start=False, stop=True)
                di = DI[d]
                nc.vector.tensor_scalar(out=cbt[s][:, di * H:(di + 1) * H],
                                        in0=h[:], scalar1=0.0, scalar2=None,
                                        op0=maxop)

            # root block: out2_R = relu(xrootT) * c_g
            rT = []
            for f in range(nF):
                t = spool.tile([P, G_cap], bf, name=f"rT{f}", tag=f"rT{f}")
                tmp = spool.tile([P, G_cap], f32, name="rtmp", tag="rtmp")
                nc.scalar.activation(tmp[:], C32(f"xrootT{f}"), Relu)
                nc.vector.tensor_tensor(out=t[:], in0=tmp[:], in1=C32("croot"),
                                        op=mul)
                rT.append(t)

            # stage 4: o2[m] [P, G_cap] = cbt[:, m-chunk]^T @ Pr
            o2_sb = []
            for m_ in range(2 * H // P):
                o2 = psB.tile([P, G_cap], f32, name="o2ps", tag="psb")
                for s in range(nS):
                    nc.tensor.matmul(out=o2[:],
                                     lhsT=cbt[s][:, m_ * P:(m_ + 1) * P],
                                     rhs=C16(f"pr{s}"), start=(s == 0),
                                     stop=(s == nS - 1))
                t = spool.tile([P, G_cap], bf, name=f"o2{m_}", tag=f"o2{m_}")
                nc.vector.tensor_scalar(out=t[:], in0=o2[:], scalar1=0.0,
                                        scalar2=None, op0=addop)
                o2_sb.append(t)

            # stage 5: totT[d] [H, G_cap] = relu(W2_d^T [rT; o2_d] + b2_d)
            tot = []
            for di, d in enumerate(("bu", "td")):
                tp = psB.tile([P, G_cap], f32, name="totps", tag="psb")
                for f in range(nW2):
                    rhs_t = rT[f] if f < nF else o2_sb[di]
                    nc.tensor.matmul(out=tp[:], lhsT=C16(f"w2{d}{f}"),
                                     rhs=rhs_t[:], start=(f == 0),
                                     stop=(f == nW2 - 1))
                t = spool.tile([P, G_cap], bf, name=f"tot{di}", tag=f"tot{di}")
                nc.vector.tensor_scalar(out=t[:], in0=tp[:],
                                        scalar1=C32(f"b2{d}"), scalar2=0.0,
                                        op0=addop, op1=maxop)
                tot.append(t)

            # stage 6: logits + log_softmax
            lg = psB.tile([G_cap, C], f32, name="lgps", tag="psb")
            nwl = 2 * H // P
            for f in range(nwl):
                nc.tensor.matmul(out=lg[:], lhsT=tot[f][:, :G_cap],
                                 rhs=C16(f"wl{f}"), start=(f == 0),
                                 stop=(bz and f == nwl - 1))
            if not bz:
                nc.tensor.matmul(out=lg[:],
                                 lhsT=C16("ones", rows=slice(0, 1))[:, :G_cap],
                                 rhs=C16("bl", rows=slice(0, 1)),
                                 start=False, stop=True)
            ez = spool.tile([G_cap, C], f32, name="ez", tag="ez")
            se = spool.tile([G_cap, 1], f32, name="se", tag="se")
            nc.scalar.activation(ez[:], lg[:], Exp, accum_out=se[:])
            lse = spool.tile([G_cap, 1], f32, name="lse", tag="lse")
            nc.scalar.activation(lse[:], se[:], Ln)
            res = spool.tile([G_cap, C], f32, name="res", tag="res")
            nc.vector.tensor_scalar(out=res[:], in0=lg[:], scalar1=lse[:],
                                    scalar2=None, op0=sub)
            nc.sync.dma_start(out_d[:], res[:])

    nc.compile()
    return nc


_PROG_CACHE = {}


def _prepare_maps(inputs):
    in_maps, meta = _preprocess(inputs["x"], inputs["edge_index"],
                                inputs["batch"], inputs["num_graphs"])
    _pack_consts(in_maps, inputs, meta)
    return in_maps, meta


def _prepare(inputs):
    in_maps, meta = _prepare_maps(inputs)
    key = (meta["F"], meta["H"], meta["C"], meta["S_cap"], meta["K"],
           meta["Kc"], meta["G_cap"], meta["bz"])
    if key not in _PROG_CACHE:
        _PROG_CACHE[key] = _build_program(*key)
    return _PROG_CACHE[key], in_maps, meta


def kernel(**inputs):
    from concourse.bass_utils import run_bass_kernel_spmd

    nc, in_maps, meta = _prepare(inputs)
    res = run_bass_kernel_spmd(nc, in_maps, list(range(NCORES)))
    G = meta["G"]
    cog, gl = meta["core_of_graph"], meta["glocal"]
    out = np.empty((G, meta["C"]), np.float32)
    for g in range(G):
        out[g] = res.results[cog[g]]["out"][gl[g]]
    return out


# revision 49
# speedup vs baseline: 1.1920x; 1.0971x over previous
"""BiGCN (nn_BiGCN_52716428591487) Trainium2 kernel.

Math: the model's output is log_softmax(cat(l2_bu[root], l2_td[root]) @ W_lin + b).
Only the layer-2 GCN values AT THE ROOT NODES matter, and GCNConv is linear in
its input features, so the whole network collapses to:

  agg1_d[v]  = sum_{e -> v} coef_d(e) * x[nbr(e)]            (v in S; self-loops
               folded into the edge list with coef dinv_d[v]^2)
  l1_d[v]    = agg1_d[v] @ W_d1 + b_d1
  out2_l1[g] = sum_{s in S_g} Pr[s, g] * relu(l1_d[s])       (layer-2 agg)
  out2_R[g]  = c_g * relu(x[root_g])                          (root-feature block
               collapses: Pr is block-diagonal by graph, c_g = sum_s Pr[s, g])
  pb/pt[g]   = relu(W2_d^T [out2_R; out2_l1_d] + b_2)
  out[g]     = log_softmax([pb, pt][g] @ W_lin + b_lin)

where S = {sources of root-incident edges} + {roots} (~1.7k of 50k nodes) and
Pr is the (structure-only) layer-2 aggregation matrix.

Host does index-only preprocessing (degrees, edge selection, gather tables,
Pr); the device does every arithmetic op that touches x: the per-edge
coefficient scaling + aggregation (host-packed fp8 one-hot matrices,
DoubleRow fp8 matmuls on the PE), all four GCN feature transforms, biases,
relus, the linear head and log_softmax.

Sharding: graph-data parallel over 8 cores; the host concatenates the
per-core [G_cap, C] outputs.
"""

import numpy as np

P = 128
NCORES = 8
CH = 4  # xt pairs per DMA



def _roundup(a, m):
    return -(-int(a) // m) * m


# ----------------------------------------------------------------------------
# Host preprocessing: index-only work + gather tables
# ----------------------------------------------------------------------------

def _preprocess(x, edge_index, batch, num_graphs):
    import concourse.mybir as mybir
    import ml_dtypes

    f8 = mybir.dt.np(mybir.dt.float8e4)
    bf16 = ml_dtypes.bfloat16

    x = np.ascontiguousarray(np.asarray(x), dtype=np.float32)
    ei = np.asarray(edge_index)
    batch = np.asarray(batch).astype(np.int64)
    G = int(np.asarray(num_graphs))
    N, F = x.shape
    src = ei[0].astype(np.int64)
    dst = ei[1].astype(np.int64)

    assert np.all(np.diff(batch) >= 0), "batch must be sorted (contiguous graphs)"
    roots = np.searchsorted(batch, np.arange(G, dtype=np.int64))  # segment_min

    deg_td = 1.0 + np.bincount(dst, minlength=N).astype(np.float64)
    deg_bu = 1.0 + np.bincount(src, minlength=N).astype(np.float64)
    dinv_td = (1.0 / np.sqrt(deg_td)).astype(np.float32)
    dinv_bu = (1.0 / np.sqrt(deg_bu)).astype(np.float32)

    G_cap = max(-(-G // NCORES), 1)

    # S: sources of root-incident edges + roots
    is_root = np.zeros(N, bool)
    is_root[roots] = True
    rmask = is_root[dst]
    r_src, r_dst = src[rmask], dst[rmask]
    r_coef = dinv_td[r_src] * dinv_td[r_dst]

    s_nodes = np.unique(np.concatenate([r_src, roots]))  # sorted
    s_graph = batch[s_nodes]
    s_count_g = np.bincount(s_graph, minlength=G)
    assert s_count_g.max() <= P, "graph S-set exceeds one chunk"

    # entry weights per graph (edges at S nodes + self loops), per direction
    in_s = np.zeros(N, bool)
    in_s[s_nodes] = True
    gw_td = np.bincount(batch[dst[in_s[dst]]], minlength=G) + s_count_g
    gw_bu = np.bincount(batch[src[in_s[src]]], minlength=G) + s_count_g

    # graph -> core: greedy balance, cap G_cap
    core_of_graph = np.empty(G, np.int64)
    glocal = np.empty(G, np.int64)
    counts = np.zeros(NCORES, np.int64)
    ld_td = np.zeros(NCORES)
    ld_bu = np.zeros(NCORES)
    for g in np.argsort(-(gw_td + gw_bu), kind="stable"):
        c = min((cc for cc in range(NCORES) if counts[cc] < G_cap),
                key=lambda cc: max(ld_td[cc] + gw_td[g], ld_bu[cc] + gw_bu[g]))
        core_of_graph[g] = c
        glocal[g] = counts[c]
        counts[c] += 1
        ld_td[c] += gw_td[g]
        ld_bu[c] += gw_bu[g]

    s_core = core_of_graph[s_graph]
    S_counts = np.bincount(s_core, minlength=NCORES)
    S_cap = max(_roundup(S_counts.max(), P), P)
    nS = S_cap // P
    assert S_cap <= 512, f"S_cap={S_cap} > 512 unsupported"

    # within each core: whole graphs -> chunks (slot bins of P), balancing
    # entry load; then S nodes get consecutive slots inside their chunk
    s_lookup = np.full(N, -1, np.int64)
    chunk_of_graph = np.full(G, -1, np.int64)
    for c in range(NCORES):
        gs = np.flatnonzero(core_of_graph == c)
        fill = np.zeros(nS, np.int64)
        loads_td = np.zeros(nS)
        loads_bu = np.zeros(nS)
        for g in gs[np.argsort(-(gw_td[gs] + gw_bu[gs]), kind="stable")]:
            b = min((bb for bb in range(nS) if fill[bb] + s_count_g[g] <= P),
                    key=lambda bb: max(loads_td[bb] + gw_td[g],
                                       loads_bu[bb] + gw_bu[g]))
            chunk_of_graph[g] = b
            idx = s_nodes[s_graph == g]
            s_lookup[idx] = b * P + fill[b] + np.arange(len(idx))
            fill[b] += s_count_g[g]
            loads_td[b] += gw_td[g]
            loads_bu[b] += gw_bu[g]

    # entry lists per (core, dir): targets in S + self loops
    def _dir_entries(tgt_nodes, row_nodes, dinv):
        m = s_lookup[tgt_nodes] >= 0
        tgt = s_lookup[tgt_nodes[m]]
        rows = row_nodes[m]
        coef = dinv[row_nodes[m]] * dinv[tgt_nodes[m]]
        core = s_core[np.searchsorted(s_nodes, tgt_nodes[m])]
        # self loops
        tgt = np.concatenate([tgt, s_lookup[s_nodes]])
        rows = np.concatenate([rows, s_nodes])
        coef = np.concatenate([coef.astype(np.float32), dinv[s_nodes] ** 2])
        core = np.concatenate([core, s_core])
        return tgt, rows, coef, core

    dirs = {"td": _dir_entries(dst, src, dinv_td),
            "bu": _dir_entries(src, dst, dinv_bu)}

    # uniform k-tile count per chunk across cores/dirs, rounded to even
    Kc = 2
    for tgt, rows, coef, core in dirs.values():
        for c in range(NCORES):
            ch_arr = tgt[core == c] // P
            for s in range(nS):
                n = int(np.count_nonzero(ch_arr == s))
                Kc = max(Kc, _roundup(-(-n // P), 2))
    K = nS * Kc

    # layer-2 aggregation matrix Pr[core, s_local, glocal] and root tables
    r_graph = batch[r_dst]
    assert np.all(core_of_graph[batch[r_src]] == core_of_graph[r_graph]), \
        "cross-core root edge unsupported"
    Pr = np.zeros((NCORES, S_cap, G_cap), np.float32)
    np.add.at(Pr, (core_of_graph[r_graph], s_lookup[r_src], glocal[r_graph]), r_coef)
    np.add.at(Pr, (core_of_graph[np.arange(G)], s_lookup[roots], glocal),
              dinv_td[roots] ** 2)

    in_maps = []
    for c in range(NCORES):
        m = {"pr": Pr[c]}
        for name, (tgt, rows, coef, core) in dirs.items():
            sel = core == c
            tg_p = np.zeros(K * P, np.float32)
            cf_p = np.zeros(K * P, np.float32)
            rows_p = np.zeros(K * P, np.int64)
            tc_, rc_, cc_ = tgt[sel], rows[sel], coef[sel]
            ch_arr = tc_ // P
            for s in range(nS):
                ss = ch_arr == s
                n = int(np.count_nonzero(ss))
                # bu processes chunks in reverse so the two directions'
                # last-chunk tail chains stagger instead of bunching
                sk = (nS - 1 - s) if name == "bu" else s
                o = sk * Kc * P
                tg_p[o:o + n] = (tc_[ss] - s * P).astype(np.float32)
                cf_p[o:o + n] = cc_[ss]
                rows_p[o:o + n] = rc_[ss]
            # host-packed fp8 one-hot pair matrices [P, K*P]
            # (pair-major: col = pair*2P + h*P + tgt), one DMA per direction
            ohm = np.zeros((K, P, P), f8)
            tg_i = tg_p.reshape(K, P).astype(np.int64)
            ohm[np.repeat(np.arange(K), P), np.tile(np.arange(P), K),
                tg_i.reshape(-1)] = cf_p.reshape(-1).astype(f8)
            m[f"oh_{name}"] = np.ascontiguousarray(
                ohm.transpose(1, 0, 2).reshape(P, K * P))
            xg = x[rows_p].astype(f8)                    # [K*P, F]
            # pair-pack for DoubleRow: [K/2, 2, P, F] -> [K/2, P, 2F], then
            # group CH pairs per DMA block: [nDMA*P, CH*2F]
            PAIRS = K // 2
            nDMA = -(-PAIRS // CH)
            xp = np.zeros((nDMA * CH, P, 2 * F), f8)
            xp[:PAIRS] = xg.reshape(PAIRS, 2, P, F).transpose(0, 2, 1, 3) \
                           .reshape(PAIRS, P, 2 * F)
            xp = np.ascontiguousarray(
                xp.reshape(nDMA, CH, P, 2 * F).transpose(0, 2, 1, 3)
                  .reshape(nDMA * P, CH * 2 * F))
            m[f"xt_{name}"] = xp
        # root tables: xrootT chunks [P, G_cap] and c_g broadcast
        gs = np.flatnonzero(core_of_graph == c)
        xrootT = np.zeros((F, G_cap), np.float32)
        xrootT[:, glocal[gs]] = x[roots[gs]].T
        m["xrootT"] = xrootT
        m["croot"] = np.tile(Pr[c].sum(axis=0, dtype=np.float64)
                             .astype(np.float32), (P, 1))
        in_maps.append(m)

    meta = dict(F=F, S_cap=S_cap, K=K, Kc=Kc, G_cap=G_cap, counts=counts, G=G,
                core_of_graph=core_of_graph, glocal=glocal)
    return in_maps, meta


def _c16_layout(F, H, C, G_cap):
    """Column layout of the bf16 constant matrix [P, W16]."""
    off = 0
    L = {}

    def add(name, w):
        nonlocal off
        L[name] = (off, w)
        off += w

    for d in ("td", "bu"):
        for f in range(F // P):
            add(f"w1{d}{f}", H)
    for d in ("bu", "td"):
        for f in range((F + H) // P):
            add(f"w2{d}{f}", H)
    for f in range(2 * H // P):
        add(f"wl{f}", C)
    add("b1td", H)
    add("b1bu", H)
    add("bl", C)
    add("ones", P)
    for s in range(2):
        add(f"pr{s}", G_cap)
    return L, off


def _c32_layout(F, G_cap):
    off = 0
    L = {}

    def add(name, w):
        nonlocal off
        L[name] = (off, w)
        off += w

    for f in range(F // P):
        add(f"xrootT{f}", G_cap)
    add("croot", G_cap)
    add("b2bu", 1)
    add("b2td", 1)
    return L, off


def _pack_consts(in_maps, inputs, meta):
    """Fold per-core constants into cold16 [P,W16] bf16 / cold32 [P,W32] fp32."""
    import ml_dtypes

    bf16 = ml_dtypes.bfloat16
    H = int(np.asarray(inputs["W_td1"]).shape[1])
    C = int(np.asarray(inputs["W_lin"]).shape[1])
    F, K, G_cap, S_cap = meta["F"], meta["K"], meta["G_cap"], meta["S_cap"]
    nS = S_cap // P
    L16, W16 = _c16_layout(F, H, C, G_cap)
    L32, W32 = _c32_layout(F, G_cap)
    g = lambda k: np.asarray(inputs[k], dtype=np.float32)

    base16 = np.zeros((P, W16), bf16)

    def put(name, block):
        o, w = L16[name]
        base16[:, o:o + w][tuple(slice(s) for s in block.shape)] = \
            block.astype(bf16)

    for d, wn in (("td", "W_td1"), ("bu", "W_bu1")):
        for f in range(F // P):
            put(f"w1{d}{f}", g(wn)[f * P:(f + 1) * P, :])
    for d, wn in (("bu", "W_bu2"), ("td", "W_td2")):
        for f in range((F + H) // P):
            put(f"w2{d}{f}", g(wn)[f * P:(f + 1) * P, :])
    for f in range(2 * H // P):
        put(f"wl{f}", g("W_lin")[f * P:(f + 1) * P, :])
    put("b1td", g("b_td1").reshape(1, H))
    put("b1bu", g("b_bu1").reshape(1, H))
    put("bl", g("b_lin").reshape(1, C))
    put("ones", np.ones((1, P), np.float32))

    for m in in_maps:
        c16 = base16.copy()
        pr = m.pop("pr")
        for s in range(nS):
            o, w = L16[f"pr{s}"]
            c16[:, o:o + w] = pr[s * P:(s + 1) * P].astype(bf16)
        m["c16"] = np.ascontiguousarray(c16)

        c32 = np.zeros((P, W32), np.float32)
        xrootT = m.pop("xrootT")
        for f in range(F // P):
            o, w = L32[f"xrootT{f}"]
            c32[:, o:o + w] = xrootT[f * P:(f + 1) * P]
        o, w = L32["croot"]
        c32[:, o:o + w] = m.pop("croot")
        c32[:, L32["b2bu"][0]] = g("b_bu2")
        c32[:, L32["b2td"][0]] = g("b_td2")
        m["c32"] = np.ascontiguousarray(c32)

    meta["H"], meta["C"] = H, C
    meta["bz"] = all(not np.any(np.asarray(inputs[k]))
                     for k in ("b_td1", "b_bu1", "b_bu2", "b_td2", "b_lin"))
    return H


# ----------------------------------------------------------------------------
# Device program
# ----------------------------------------------------------------------------

def _build_program(F, H, C, S_cap, K, Kc, G_cap, bz=False, repeat=1):
    from contextlib import ExitStack

    import concourse.bacc as bacc
    from concourse.masks import make_identity
    import concourse.bass as bass  # noqa: F401
    import concourse.mybir as mybir
    import concourse.tile as tile

    dt = mybir.dt
    f32, bf, f8 = dt.float32, dt.bfloat16, dt.float8e4
    nF = F // P
    nS = S_cap // P
    nW2 = (F + H) // P
    PAIRS = K // 2
    PPC = Kc // 2                   # pairs per chunk
    nDMA = -(-PAIRS // CH)          # xt DMAs per direction
    assert K == nS * Kc and Kc % 2 == 0
    assert F % P == 0 and H == P and (F + H) % P == 0
    L16, W16 = _c16_layout(F, H, C, G_cap)
    L32, W32 = _c32_layout(F, G_cap)
    WH = P + 4 * K

    nc = bacc.Bacc("TRN2", target_bir_lowering=False, debug=False,
                   num_devices=NCORES)

    xt_d = {d: nc.dram_tensor(f"xt_{d}", [nDMA * P, CH * 2 * F], f8,
                              kind="ExternalInput").ap() for d in ("td", "bu")}
    oh_d = {d: nc.dram_tensor(f"oh_{d}", [P, K * P], f8,
                              kind="ExternalInput").ap() for d in ("td", "bu")}
    c16_d = nc.dram_tensor("c16", [P, W16], bf, kind="ExternalInput").ap()
    c32_d = nc.dram_tensor("c32", [P, W32], f32, kind="ExternalInput").ap()
    out_d = nc.dram_tensor("out", [G_cap, C], f32, kind="ExternalOutput").ap()

    eq, mul, sub, addop, maxop = (
        mybir.AluOpType.is_equal, mybir.AluOpType.mult,
        mybir.AluOpType.subtract, mybir.AluOpType.add, mybir.AluOpType.max)
    Relu, Exp, Ln, Copy = (mybir.ActivationFunctionType.Relu,
                           mybir.ActivationFunctionType.Exp,
                           mybir.ActivationFunctionType.Ln,
                           mybir.ActivationFunctionType.Copy)
    DR = mybir.MatmulPerfMode.DoubleRow

    with ExitStack() as ctx:
        tc = ctx.enter_context(tile.TileContext(nc))
        fixed = ctx.enter_context(tc.tile_pool(name="fx", bufs=1))
        const = ctx.enter_context(tc.tile_pool(
            name="cst", bufs=(1 if repeat == 1 else 2)))
        xpool = ctx.enter_context(tc.tile_pool(name="xp", bufs=8))
        ohpool = ctx.enter_context(tc.tile_pool(name="oh", bufs=2))
        apool = ctx.enter_context(tc.tile_pool(name="ap", bufs=4))
        tpool = ctx.enter_context(tc.tile_pool(name="tp", bufs=2))
        spool = ctx.enter_context(tc.tile_pool(name="sp", bufs=2))
        psA = ctx.enter_context(tc.tile_pool(name="psA", bufs=1, space="PSUM"))
        psB = ctx.enter_context(tc.tile_pool(name="psB", bufs=4, space="PSUM"))

        ident = fixed.tile([P, P], bf, name="ident", tag="ident")
        make_identity(nc, ident[:])
        # load the one act table containing Exp/Ln/Relu/Copy up-front
        # (overlaps the initial DMA waits); the insert_act_table_loads pass
        # then has nothing to add mid-program
        from concourse.hw_specs import get_activation_tables
        need = {Exp, Ln, Relu, Copy}
        for set_id, funcs in enumerate(get_activation_tables(nc.m.arch).values()):
            if need <= funcs:
                nc.scalar.add_instruction(mybir.InstLoadActFuncSet(
                    name=nc.get_next_instruction_name(),
                    act_func_set_id=set_id, ins=[], outs=[]))
                break

        for _rep in range(repeat):
            c16 = const.tile([P, W16], bf, name="c16", tag="c16")
            c32 = const.tile([P, W32], f32, name="c32", tag="c32")

            def C16(name, rows=None):
                o, w = L16[name]
                return c16[:, o:o + w] if rows is None else c16[rows, o:o + w]

            def C32(name, rows=None):
                o, w = L32[name]
                return c32[:, o:o + w] if rows is None else c32[rows, o:o + w]

            # one-hot matrices (one DMA per direction) + xt stream
            ohm = {}
            xtiles = {"td": [], "bu": []}
            for d in ("td", "bu"):
                ohm[d] = ohpool.tile([P, K * P], f8, name=f"ohm{d}",
                                     tag=f"ohm{d}")
                nc.sync.dma_start(ohm[d][:], oh_d[d][:, :])
            for ci in range(nDMA):
                if ci == (nDMA + 1) // 2:
                    nc.sync.dma_start(c16[:], c16_d[:, :])
                    nc.sync.dma_start(c32[:], c32_d[:, :])
                for d in ("td", "bu"):
                    npair = min(CH, PAIRS - ci * CH)
                    t = xpool.tile([P, CH * 2 * F], f8, name="xt", tag="xt")
                    nc.sync.dma_start(
                        t[:, :npair * 2 * F],
                        xt_d[d][ci * P:(ci + 1) * P, :npair * 2 * F])
                    xtiles[d].append(t)

            # stage 1: DoubleRow fp8 matmuls with host-packed one-hot pairs
            agg_ps = {d: [psA.tile([P, F], f32, name=f"agg{d}{s}",
                                   tag=f"agg{d}{s}") for s in range(nS)]
                      for d in ("td", "bu")}
            done = []   # (d, s) chunks completed, in order
            for ci in range(nDMA):
                for d in ("td", "bu"):
                    npair = min(CH, PAIRS - ci * CH)
                    for j in range(npair):
                        pr_i = ci * CH + j
                        s = pr_i // PPC
                        if d == "bu":
                            s = nS - 1 - s
                        nc.tensor.matmul(
                            out=agg_ps[d][s][:],
                            lhsT=ohm[d][:, pr_i * 2 * P:(pr_i + 1) * 2 * P]
                                .rearrange("p (a b) -> p a b", a=2),
                            rhs=xtiles[d][ci][:, j * 2 * F:(j + 1) * 2 * F]
                                .rearrange("p (a b) -> p a b", a=2),
                            start=(pr_i % PPC == 0),
                            stop=(pr_i % PPC == PPC - 1),
                            perf_mode=DR)
                        if pr_i % PPC == PPC - 1:
                            done.append((d, s))

            # stage 1.5 + 2, per completed (d, s) chunk:
            # psum -> sbuf (Act), transpose (PE), copy back (Act),
            # l1 = aggT^T @ W1 + b1 (PE), relu -> cbt (DVE)
            aggT = {d: [tpool.tile([P, S_cap], bf, name=f"aT{d}{f}",
                                   tag=f"aT{d}{f}") for f in range(nF)]
                    for d in ("td", "bu")}
            cbt = [spool.tile([P, 2 * H], bf, name=f"cbt{s}", tag=f"cbt{s}")
                   for s in range(nS)]
            DI = {"bu": 0, "td": 1}
            for d, s in done:
                # the last chunks' chains are tail-critical: run one on DVE
                # (idle after the one-hot builds) so they parallelize with
                # the Activation-engine chain of the other
                on_dve = s == nS - 1 and d == "td"

                def _copy(dst, src_ap):
                    if on_dve:
                        nc.vector.tensor_scalar(out=dst, in0=src_ap,
                                                scalar1=0.0, scalar2=None,
                                                op0=addop)
                    else:
                        nc.scalar.activation(dst, src_ap, Copy)

                aggS = apool.tile([P, F], bf, name="aggS", tag="aggS")
                _copy(aggS[:], agg_ps[d][s][:])
                for f in range(nF):
                    tps = psB.tile([P, P], bf, name="tps", tag="psb")
                    nc.tensor.transpose(out=tps[:],
                                        in_=aggS[:, f * P:(f + 1) * P],
                                        identity=ident[:])
                    _copy(aggT[d][f][:, s * P:(s + 1) * P], tps[:])
                h = psB.tile([P, H], f32, name="hps", tag="psb")
                for f in range(nF):
                    nc.tensor.matmul(out=h[:],
                                     lhsT=aggT[d][f][:, s * P:(s + 1) * P],
                                     rhs=C16(f"w1{d}{f}"),
                                     start=(f == 0),
                                     stop=(bz and f == nF - 1))
                if not bz:
                    nc.tensor.matmul(out=h[:],
                                     lhsT=C16("ones", rows=slice(0, 1)),
                                     rhs=C16(f"b1{d}", rows=slice(0, 1)),
                                     start=False, stop=True)
                di = DI[d]
                nc.vector.tensor_scalar(out=cbt[s][:, di * H:(di + 1) * H],
                                        in0=h[:], scalar1=0.0, scalar2=None,
                                        op0=maxop)

            # root block: out2_R = relu(xrootT) * c_g
            rT = []
            for f in range(nF):
                t = spool.tile([P, G_cap], bf, name=f"rT{f}", tag=f"rT{f}")
                tmp = spool.tile([P, G_cap], f32, name="rtmp", tag="rtmp")
                nc.scalar.activation(tmp[:], C32(f"xrootT{f}"), Relu)
                nc.vector.tensor_tensor(out=t[:], in0=tmp[:], in1=C32("croot"),
                                        op=mul)
                rT.append(t)

            # stage 4: o2[m] [P, G_cap] = cbt[:, m-chunk]^T @ Pr
            o2_sb = []
            for m_ in range(2 * H // P):
                o2 = psB.tile([P, G_cap], f32, name="o2ps", tag="psb")
                for s in range(nS):
                    nc.tensor.matmul(out=o2[:],
                                     lhsT=cbt[s][:, m_ * P:(m_ + 1) * P],
                                     rhs=C16(f"pr{s}"), start=(s == 0),
                                     stop=(s == nS - 1))
                t = spool.tile([P, G_cap], bf, name=f"o2{m_}", tag=f"o2{m_}")
                nc.vector.tensor_scalar(out=t[:], in0=o2[:], scalar1=0.0,
                                        scalar2=None, op0=addop)
                o2_sb.append(t)

            # stage 5: totT[d] [H, G_cap] = relu(W2_d^T [rT; o2_d] + b2_d)
            tot = []
            for di, d in enumerate(("bu", "td")):
                tp = psB.tile([P, G_cap], f32, name="totps", tag="psb")
                for f in range(nW2):
                    rhs_t = rT[f] if f < nF else o2_sb[di]
                    nc.tensor.matmul(out=tp[:], lhsT=C16(f"w2{d}{f}"),
                                     rhs=rhs_t[:], start=(f == 0),
                                     stop=(f == nW2 - 1))
                t = spool.tile([P, G_cap], bf, name=f"tot{di}", tag=f"tot{di}")
                nc.vector.tensor_scalar(out=t[:], in0=tp[:],
                                        scalar1=C32(f"b2{d}"), scalar2=0.0,
                                        op0=addop, op1=maxop)
                tot.append(t)

            # stage 6: logits + log_softmax
            lg = psB.tile([G_cap, C], f32, name="lgps", tag="psb")
            nwl = 2 * H // P
            for f in range(nwl):
                nc.tensor.matmul(out=lg[:], lhsT=tot[f][:, :G_cap],
                                 rhs=C16(f"wl{f}"), start=(f == 0),
                                 stop=(bz and f == nwl - 1))
            if not bz:
                nc.tensor.matmul(out=lg[:],
                                 lhsT=C16("ones", rows=slice(0, 1))[:, :G_cap],
                                 rhs=C16("bl", rows=slice(0, 1)),
                                 start=False, stop=True)
            ez = spool.tile([G_cap, C], f32, name="ez", tag="ez")
            se = spool.tile([G_cap, 1], f32, name="se", tag="se")
            nc.scalar.activation(ez[:], lg[:], Exp, accum_out=se[:])
            lse = spool.tile([G_cap, 1], f32, name="lse", tag="lse")
            nc.scalar.activation(lse[:], se[:], Ln)
            res = spool.tile([G_cap, C], f32, name="res", tag="res")
            nc.vector.tensor_scalar(out=res[:], in0=lg[:], scalar1=lse[:],
                                    scalar2=None, op0=sub)
            nc.sync.dma_start(out_d[:], res[:])

    nc.compile()
    return nc


_PROG_CACHE = {}


def _prepare_maps(inputs):
    in_maps, meta = _preprocess(inputs["x"], inputs["edge_index"],
                                inputs["batch"], inputs["num_graphs"])
    _pack_consts(in_maps, inputs, meta)
    return in_maps, meta


def _prepare(inputs):
    in_maps, meta = _prepare_maps(inputs)
    key = (meta["F"], meta["H"], meta["C"], meta["S_cap"], meta["K"],
           meta["Kc"], meta["G_cap"], meta["bz"])
    if key not in _PROG_CACHE:
        _PROG_CACHE[key] = _build_program(*key)
    return _PROG_CACHE[key], in_maps, meta


def kernel(**inputs):
    from concourse.bass_utils import run_bass_kernel_spmd

    nc, in_maps, meta = _prepare(inputs)
    res = run_bass_kernel_spmd(nc, in_maps, list(range(NCORES)))
    G = meta["G"]
    cog, gl = meta["core_of_graph"], meta["glocal"]
    out = np.empty((G, meta["C"]), np.float32)
    for g in range(G):
        out[g] = res.results[cog[g]]["out"][gl[g]]
    return out
